# revision 100
# baseline (speedup 1.0000x reference)
"""MPNN (2x NNConv + BN + global mean pool + MLP) on 8 Trainium2 cores.

Strategy (node-sharded message passing, v2):
  * Never materialize We=[E,in_c,out_c].  msg[e] = (z[e] (x) xs[e]) @ W2r
    where z=relu(ea@W1+b1), xs=x[src], W2r = reshape of W2.  Since the
    segment-sum over dst commutes with the (shared) @W2r, we scatter the
    per-edge outer products u[e]=(z (x) xs_scaled) into per-node U first,
    then do ONE matmul per node tile:  agg = U @ W2r.
  * Nodes are bin-packed into 128-node windows balanced by edge count;
    each core owns 10 windows.  Edges are grouped per window (padded to
    T tiles of 128).  Scatter = one-hot matmul on the PE, per 128-column
    chunk, pipelined scatter(PE) -> cast(Scalar) -> agg(PE) with small
    double-buffered PSUM tiles.  Operands bf16, PSUM accumulates f32.
  * v2 changes vs v1: host pre-gathers x[src]*icnt for layer 1 (no
    indirect DMA in L1); one-hot scatter/pool matrices built on host and
    DMA'd once (reused by both layers); h1 AllGathered in bf16 with BN
    stats riding along as hi/lo bf16 row pairs; layer-2 edge-MLP (z) and
    the h1 transposes computed while the AllGather is in flight; BN1
    folded into root2/bias2 algebraically; outer products split across
    Vector and GpSimd; BN stats accumulate in PSUM (L1) or ride the pool
    matmul (L2); collective outputs in Shared DRAM.
"""

import sys

import numpy as np

try:
    import concourse.bass as bass  # noqa
except Exception:  # pragma: no cover
    sys.path.insert(0, "/opt/trn_rl_repo")

import ml_dtypes
import concourse.bacc as bacc
import concourse.bass as bass
import concourse.mybir as mybir
import concourse.tile as tile
from concourse.bass import IndirectOffsetOnAxis
from concourse.bass_utils import run_bass_kernel_spmd

P = 128
NCORES = 8
N = 10000
E = 30000
NG = 256
IN_C = 16
EDGE_C = 8
KH = 32  # edge-MLP hidden width
H1 = 64  # conv1 out channels
H2 = 128  # conv2 out channels
WPC = 10  # windows per core
NPADC = WPC * P  # padded nodes per core (1280)
NSTR = NPADC + 4  # h1 rows per core incl. 4 BN-stats rows (hi/lo)
EPS = 1e-5
f32 = mybir.dt.float32
bf16 = mybir.dt.bfloat16
i32 = mybir.dt.int32
bf = ml_dtypes.bfloat16

U1 = KH * IN_C  # 512
U1R = U1 + IN_C  # 528 = 4 full chunks + a 16-wide tail chunk (the b2 block)
U2 = KH * H1  # 2048
U2R = U2 + H1  # 2112 = 16 full chunks + a 64-wide tail chunk
NC1 = -(-U1R // P)  # 5
NC2 = -(-U2R // P)  # 17
# outer-product k-split between Vector and GpSimd (multiples of 2 so the
# engine boundary lands on a 128-column chunk boundary)
KV2 = 28  # L2: vector does k<28 (14 chunks), gpsimd k>=28 (2 chunks)
UBUFS = 6


# --------------------------------------------------------------------------
# host-side preprocessing: index/layout work only
# --------------------------------------------------------------------------
def _preprocess(x, edge_index, edge_attr, batch):
    import heapq

    src = np.asarray(edge_index[0], dtype=np.int64)
    dst = np.asarray(edge_index[1], dtype=np.int64)
    deg = np.bincount(dst, minlength=N).astype(np.int64)

    # ---- bin-pack nodes into NCORES*WPC windows of exactly <=128 nodes,
    # balancing per-window edge counts (LPT greedy) ----
    NW = NCORES * WPC
    order = np.argsort(-deg, kind="stable")
    wsum = np.zeros(NW, dtype=np.int64)
    wcnt = np.zeros(NW, dtype=np.int64)
    win_of = np.empty(N, dtype=np.int64)
    slot_of = np.empty(N, dtype=np.int64)
    heap = [(0, w) for w in range(NW)]
    heapq.heapify(heap)
    for n in order:
        while True:
            _, w = heapq.heappop(heap)
            if wcnt[w] < P:
                break
        win_of[n] = w
        slot_of[n] = wcnt[w]
        wcnt[w] += 1
        wsum[w] += deg[n]
        if wcnt[w] < P:
            heapq.heappush(heap, (int(wsum[w]), w))

    T = max(1, int(-(-int(wsum.max()) // P)))  # tiles (of 128 edges) per window
    ES = WPC * T * P  # edge slots per core
    TT = WPC * T  # tiles per core

    core_of = win_of // WPC
    lpos = (win_of % WPC) * P + slot_of

    # ---- per-edge placement ----
    ew = win_of[dst]
    eorder = np.argsort(ew, kind="stable")
    inv_cnt = 1.0 / np.maximum(deg, 1).astype(np.float32)

    ea_s = np.zeros((NCORES, ES, EDGE_C), dtype=np.float32)
    ones_s = np.zeros((NCORES, ES), dtype=np.float32)
    srcx_s = np.zeros((NCORES, ES), dtype=np.int64)
    srch_s = np.zeros((NCORES, ES), dtype=np.int32)
    dstrel_s = np.full((NCORES, ES), -1, dtype=np.int64)
    icnt_s = np.zeros((NCORES, ES), dtype=np.float32)

    ew_sorted = ew[eorder]
    starts = np.searchsorted(ew_sorted, np.arange(NW))
    ends = np.searchsorted(ew_sorted, np.arange(NW) + 1)
    ea_np = np.asarray(edge_attr, dtype=np.float32)
    for w in range(NW):
        es = eorder[starts[w] : ends[w]]
        c = w // WPC
        base = (w % WPC) * T * P
        k = len(es)
        assert k <= T * P
        sl = slice(base, base + k)
        ea_s[c, sl] = ea_np[es]
        ones_s[c, sl] = 1.0
        srcx_s[c, sl] = src[es]
        srch_s[c, sl] = (core_of[src[es]] * NSTR + lpos[src[es]]).astype(np.int32)
        dstrel_s[c, sl] = slot_of[dst[es]]
        icnt_s[c, sl] = inv_cnt[dst[es]]

    eaT = np.concatenate(
        [np.transpose(ea_s, (0, 2, 1)), ones_s[:, None, :]], axis=1
    ).astype(bf)  # [NC, 9, ES]

    # pre-gathered, count-scaled L1 source features, tile-major layout
    x_np = np.asarray(x, dtype=np.float32)
    xss1 = (x_np[srcx_s.reshape(-1)] * icnt_s.reshape(-1)[:, None]).reshape(
        NCORES, TT, P, IN_C
    )
    xss1 = np.ascontiguousarray(np.transpose(xss1, (0, 2, 1, 3)).astype(bf))

    # dst slot per edge, tile-major (-1 for pads; 0..127 are exact in bf16)
    drel = np.ascontiguousarray(
        dstrel_s.reshape(NCORES, TT, P).transpose(0, 2, 1).astype(np.float32)
    )

    # ---- per-node per-core tables ----
    batch = np.asarray(batch, dtype=np.int64)
    gcnt = np.bincount(batch, minlength=NG).astype(np.int64)
    igc_node = (1.0 / np.maximum(gcnt, 1).astype(np.float32))[batch]

    xT_s = np.zeros((NCORES, IN_C, NPADC), dtype=np.float32)
    vmask_s = np.zeros((NCORES, NPADC), dtype=np.float32)
    batchrel = np.full((NCORES, NPADC), -1.0, dtype=np.float32)
    igcw = np.zeros((NCORES, NPADC), dtype=np.float32)
    for c in range(NCORES):
        mc = core_of == c
        xT_s[c][:, lpos[mc]] = x_np[mc].T
        vmask_s[c][lpos[mc]] = 1.0
        batchrel[c][lpos[mc]] = batch[mc].astype(np.float32)
        igcw[c][lpos[mc]] = igc_node[mc]
    vmaskw = np.ascontiguousarray(
        vmask_s.reshape(NCORES, WPC, P).transpose(0, 2, 1)
    )  # [NC, P, WPC] f32
    batchrelw = np.ascontiguousarray(
        batchrel.reshape(NCORES, WPC, P).transpose(0, 2, 1)
    )
    igcww = np.ascontiguousarray(
        igcw.reshape(NCORES, WPC, P).transpose(0, 2, 1)
    )

    return dict(
        T=T, ES=ES, TT=TT, eaT=eaT, srch=srch_s, icnt=icnt_s, xss1=xss1,
        drel=drel, xT=xT_s, vmaskw=vmaskw, vmaskb=vmaskw.astype(bf),
        batchrelw=batchrelw, igcww=igcww,
    )


def _weights(p):
    w = {}
    w["W1a1"] = np.concatenate([p["nn1_W1"], p["nn1_b1"][None, :]], 0).astype(bf)
    w["W1a2"] = np.concatenate([p["nn2_W1"], p["nn2_b1"][None, :]], 0).astype(bf)
    wp1 = np.zeros((NC1 * P, H1), dtype=np.float32)
    wp1[:U1] = p["nn1_W2"].reshape(KH, IN_C, H1).reshape(U1, H1)
    wp1[U1:U1R] = p["nn1_b2"].reshape(IN_C, H1)
    w["Wp1"] = np.ascontiguousarray(
        wp1.reshape(NC1, P, H1).transpose(1, 0, 2).astype(bf)
    )  # [P, NC1, H1]
    wp2 = np.zeros((NC2 * P, H2), dtype=np.float32)
    wp2[:U2] = p["nn2_W2"].reshape(KH, H1, H2).reshape(U2, H2)
    wp2[U2:U2R] = p["nn2_b2"].reshape(H1, H2)
    w["Wp2"] = np.ascontiguousarray(
        wp2.reshape(NC2, P, H2).transpose(1, 0, 2).astype(bf)
    )  # [P, NC2, H2]
    w["root1"] = np.asarray(p["root1"], np.float32).astype(bf)
    w["root2"] = np.asarray(p["root2"], np.float32).astype(bf)
    w["bias1r"] = np.asarray(p["bias1"], np.float32)[None, :].astype(bf)
    w["bias2r"] = np.asarray(p["bias2"], np.float32)[None, :].astype(bf)
    w["bng1r"] = np.asarray(p["bn1_g"], np.float32)[None, :]
    w["bnb1r"] = np.asarray(p["bn1_b"], np.float32)[None, :]
    w["bng2"] = np.asarray(p["bn2_g"], np.float32)[:, None]
    w["bnb2"] = np.asarray(p["bn2_b"], np.float32)[:, None]
    w["l1W"] = np.asarray(p["lin1_W"], np.float32)
    w["l1b"] = np.asarray(p["lin1_b"], np.float32)[:, None]
    w["l2W"] = np.asarray(p["lin2_W"], np.float32)
    w["l2b"] = np.asarray(p["lin2_b"], np.float32)[None, :]
    w["identb"] = np.eye(P, dtype=np.float32).astype(bf)
    w["onesr"] = np.ones((1, P), dtype=np.float32)
    w["onesrb"] = np.ones((1, P), dtype=bf)
    w["onesPb"] = np.ones((P, 1), dtype=bf)
    w["iota128"] = np.broadcast_to(
        np.arange(P, dtype=np.float32), (P, P)).astype(bf).copy()
    w["iota256"] = np.broadcast_to(
        np.arange(NG, dtype=np.float32), (P, NG)).astype(bf).copy()
    # [i, k, o] view of the conv2 edge-MLP output weights (+ b2 as k=KH),
    # used to build V[k,o] = sum_i sh1_i * W2[k,i,o] on device
    w2kx = np.zeros((H1, KH + 1, H2), dtype=np.float32)
    w2kx[:, :KH, :] = p["nn2_W2"].reshape(KH, H1, H2).transpose(1, 0, 2)
    w2kx[:, KH, :] = p["nn2_b2"].reshape(H1, H2)
    w["W2kx"] = np.ascontiguousarray(w2kx.reshape(H1, (KH + 1) * H2).astype(bf))
    # dup64[i, p] = 1 if p % 64 == i: broadcasts sc_c [64] to [128]
    w["dup64"] = np.ascontiguousarray(
        (np.arange(P)[None, :] % H1 == np.arange(H1)[:, None]).astype(bf)
    )
    return w


# --------------------------------------------------------------------------
# device program (identical for all cores; per-core data comes via inputs)
# --------------------------------------------------------------------------
DBG = False


def build_program(T, ES):
    TT = WPC * T
    AL = mybir.AluOpType
    nc = bacc.Bacc("TRN2", target_bir_lowering=False, debug=False, num_devices=NCORES)

    def din(name, shape, dtype=f32):
        return nc.dram_tensor(name, shape, dtype, kind="ExternalInput").ap()

    eaT_d = din("eaT", [EDGE_C + 1, ES], bf16)
    xss1_d = din("xss1", [P, TT, IN_C], bf16)
    drel_d = din("drel", [P, TT])
    srch_d = din("srch", [P, TT], i32)
    icnt_d = din("icnt", [P, TT])
    xT_d = din("xT", [IN_C, NPADC], bf16)
    vmaskw_d = din("vmaskw", [P, WPC])
    vmaskb_d = din("vmaskb", [P, WPC], bf16)
    batchrelw_d = din("batchrelw", [P, WPC])
    igcww_d = din("igcww", [P, WPC])
    iota128_d = din("iota128", [P, P], bf16)
    iota256_d = din("iota256", [P, NG], bf16)
    W1a1_d = din("W1a1", [EDGE_C + 1, KH], bf16)
    W1a2_d = din("W1a2", [EDGE_C + 1, KH], bf16)
    Wp1_d = din("Wp1", [P, NC1, H1], bf16)
    Wp2_d = din("Wp2", [P, NC2, H2], bf16)
    root1_d = din("root1", [IN_C, H1], bf16)
    root2_d = din("root2", [H1, H2], bf16)
    bias1r_d = din("bias1r", [1, H1], bf16)
    bias2r_d = din("bias2r", [1, H2], bf16)
    bng1r_d = din("bng1r", [1, H1])
    bnb1r_d = din("bnb1r", [1, H1])
    bng2_d = din("bng2", [H2, 1])
    bnb2_d = din("bnb2", [H2, 1])
    l1W_d = din("l1W", [H2, H1])
    l1b_d = din("l1b", [H1, 1])
    l2W_d = din("l2W", [H1, 1])
    l2b_d = din("l2b", [1, 1])
    identb_d = din("identb", [P, P], bf16)
    onesr_d = din("onesr", [1, P])
    onesrb_d = din("onesrb", [1, P], bf16)
    onesPb_d = din("onesPb", [P, 1], bf16)
    W2kx_d = din("W2kx", [H1, (KH + 1) * H2], bf16)
    dup64_d = din("dup64", [H1, P], bf16)
    out_d = nc.dram_tensor("out", [1, NG], f32, kind="ExternalOutput").ap()
    if DBG:
        dbg_h1 = nc.dram_tensor("dbg_h1", [P, 2 * H1], f32, kind="ExternalOutput").ap()
        dbg_xs = nc.dram_tensor("dbg_xs", [P, H1], f32, kind="ExternalOutput").ap()
        dbg_z = nc.dram_tensor("dbg_z", [P, KH], f32, kind="ExternalOutput").ap()
        dbg_st = nc.dram_tensor("dbg_st", [1, 4 * H1], f32, kind="ExternalOutput").ap()
        dbg_finl = nc.dram_tensor("dbg_finl", [P, NG + 2], f32, kind="ExternalOutput").ap()
        dbg_h2 = nc.dram_tensor("dbg_h2", [P, H2], f32, kind="ExternalOutput").ap()
        dbg_bn = nc.dram_tensor("dbg_bn", [1, 2 * H1], f32, kind="ExternalOutput").ap()
        dbg_xss = nc.dram_tensor("dbg_xss", [P, H1], f32, kind="ExternalOutput").ap()
        dbg_u = nc.dram_tensor("dbg_u", [P, H1], f32, kind="ExternalOutput").ap()
        dbg_uts = nc.dram_tensor("dbg_uts", [P, P], f32, kind="ExternalOutput").ap()
        dbg_fin = nc.dram_tensor("dbg_fin", [P, NG + 2], f32, kind="ExternalOutput").ap()

    from contextlib import ExitStack

    with tile.TileContext(nc) as tc, ExitStack() as pools:
        cst = pools.enter_context(tc.tile_pool(name="cst", bufs=1))
        sb = pools.enter_context(tc.tile_pool(name="sb", bufs=3))
        stash = pools.enter_context(tc.tile_pool(name="stash", bufs=WPC))
        pp = pools.enter_context(tc.tile_pool(name="pp", bufs=1, space="PSUM"))
        dram = pools.enter_context(tc.tile_pool(name="dram", bufs=1, space="DRAM"))

        # ---- resident data, loaded once: L1-critical first, in window order,
        # then everything only needed later (L2/BN/tail) ----
        W1a1 = cst.tile([EDGE_C + 1, KH], bf16, tag="W1a1")
        nc.sync.dma_start(out=W1a1[:], in_=W1a1_d[:])
        Wp1 = cst.tile([P, NC1, H1], bf16, tag="Wp1")
        nc.sync.dma_start(out=Wp1[:], in_=Wp1_d[:])
        xT = cst.tile([IN_C, NPADC], bf16, tag="xT")
        nc.sync.dma_start(out=xT[:], in_=xT_d[:])
        root1 = cst.tile([IN_C, H1], bf16, tag="root1")
        nc.sync.dma_start(out=root1[:], in_=root1_d[:])
        bias1r = cst.tile([1, H1], bf16, tag="bias1r")
        nc.sync.dma_start(out=bias1r[:], in_=bias1r_d[:])
        vmaskw = cst.tile([P, WPC], f32, tag="vmaskw")
        nc.sync.dma_start(out=vmaskw[:], in_=vmaskw_d[:])
        onesPb = cst.tile([P, 1], bf16, tag="onesPb")
        nc.sync.dma_start(out=onesPb[:], in_=onesPb_d[:])
        onesrb = cst.tile([1, P], bf16, tag="onesrb")
        nc.sync.dma_start(out=onesrb[:], in_=onesrb_d[:])
        iota128 = cst.tile([P, P], bf16, tag="iota128")
        nc.sync.dma_start(out=iota128[:], in_=iota128_d[:])
        drel = cst.tile([P, TT], f32, tag="drel")
        nc.sync.dma_start(out=drel[:], in_=drel_d[:])

        eaT = cst.tile([EDGE_C + 1, ES], bf16, tag="eaT")
        oh = cst.tile([P, TT, P], bf16, tag="oh")
        xss1 = cst.tile([P, TT, IN_C], bf16, tag="xss1")
        for w in range(WPC):
            sl = slice(w * T * P, (w + 1) * T * P)
            nc.sync.dma_start(out=eaT[:, sl], in_=eaT_d[:, sl])
            nc.sync.dma_start(out=xss1[:, w * T : (w + 1) * T, :],
                              in_=xss1_d[:, w * T : (w + 1) * T, :])

        # --- needed only from the AllGather phase on ---
        W1a2 = cst.tile([EDGE_C + 1, KH], bf16, tag="W1a2")
        nc.sync.dma_start(out=W1a2[:], in_=W1a2_d[:])
        identb = cst.tile([P, P], bf16, tag="identb")
        nc.sync.dma_start(out=identb[:], in_=identb_d[:])
        onesr = cst.tile([1, P], f32, tag="onesr")
        nc.sync.dma_start(out=onesr[:], in_=onesr_d[:])
        root2 = cst.tile([H1, H2], bf16, tag="root2")
        nc.sync.dma_start(out=root2[:], in_=root2_d[:])
        bias2r = cst.tile([1, H2], bf16, tag="bias2r")
        nc.sync.dma_start(out=bias2r[:], in_=bias2r_d[:])
        srch = cst.tile([P, TT], i32, tag="srch")
        nc.sync.dma_start(out=srch[:], in_=srch_d[:])
        icnt = cst.tile([P, TT], f32, tag="icnt")
        nc.sync.dma_start(out=icnt[:], in_=icnt_d[:])
        bng1r = cst.tile([1, H1], f32, tag="bng1r")
        nc.sync.dma_start(out=bng1r[:], in_=bng1r_d[:])
        bnb1r = cst.tile([1, H1], f32, tag="bnb1r")
        nc.sync.dma_start(out=bnb1r[:], in_=bnb1r_d[:])
        vmaskb = cst.tile([P, WPC], bf16, tag="vmaskb")
        nc.sync.dma_start(out=vmaskb[:], in_=vmaskb_d[:])
        batchrelw = cst.tile([P, WPC], f32, tag="batchrelw")
        nc.sync.dma_start(out=batchrelw[:], in_=batchrelw_d[:])
        igcww = cst.tile([P, WPC], f32, tag="igcww")
        nc.sync.dma_start(out=igcww[:], in_=igcww_d[:])
        iota256 = cst.tile([P, NG], bf16, tag="iota256")
        nc.sync.dma_start(out=iota256[:], in_=iota256_d[:])
        Wp2 = cst.tile([P, NC2, H2], bf16, tag="Wp2")
        for cq in range(NC2):
            nc.sync.dma_start(out=Wp2[:, cq, :], in_=Wp2_d[:, cq, :])
        bng2 = cst.tile([H2, 1], f32, tag="bng2")
        nc.sync.dma_start(out=bng2[:], in_=bng2_d[:])
        bnb2 = cst.tile([H2, 1], f32, tag="bnb2")
        nc.sync.dma_start(out=bnb2[:], in_=bnb2_d[:])
        l1W = cst.tile([H2, H1], f32, tag="l1W")
        nc.sync.dma_start(out=l1W[:], in_=l1W_d[:])
        l1b = cst.tile([H1, 1], f32, tag="l1b")
        nc.sync.dma_start(out=l1b[:], in_=l1b_d[:])
        l2W = cst.tile([H1, 1], f32, tag="l2W")
        nc.sync.dma_start(out=l2W[:], in_=l2W_d[:])
        l2b = cst.tile([1, 1], f32, tag="l2b")
        nc.sync.dma_start(out=l2b[:], in_=l2b_d[:])
        W2kx = cst.tile([H1, (KH + 1) * H2], bf16, tag="W2kx")
        nc.sync.dma_start(out=W2kx[:], in_=W2kx_d[:])
        dup64 = cst.tile([H1, P], bf16, tag="dup64")
        nc.sync.dma_start(out=dup64[:], in_=dup64_d[:])

        h1_slice = dram.tile([NSTR, H1], bf16, tag="h1s")
        h1_full = dram.tile([NCORES * NSTR, H1], bf16, tag="h1f")
        fin_loc = dram.tile([P, NG + 2], f32, tag="finl")
        fin_g = dram.tile([P, NG + 2], f32, tag="fing")

        groups = [list(range(NCORES))]

        # ==================== layer 1 ====================
        # scatter one-hots for all tiles, built up-front (vector is otherwise
        # idle while window 0's edge data streams in); reused by layer 2
        for t in range(TT):
            nc.vector.tensor_scalar(
                out=oh[:, t, :], in0=iota128[:], scalar1=drel[:, t : t + 1],
                scalar2=None, op0=AL.is_equal,
            )
        h1b_list = []
        for w in range(WPC):
            u_tiles = []
            for t3 in range(T):
                t = w * T + t3
                s0 = t * P
                zp = pp.tile([P, KH], f32, tag="z", bufs=2)
                nc.tensor.matmul(out=zp[:], lhsT=eaT[:, s0 : s0 + P], rhs=W1a1[:],
                                 start=True, stop=True)
                z1 = sb.tile([P, KH], bf16, tag="z1")
                nc.vector.tensor_scalar_max(out=z1[:], in0=zp[:], scalar1=0.0)

                u = sb.tile([P, U1R], bf16, tag="u1", bufs=UBUFS)
                nc.vector.tensor_tensor(
                    out=u[:, :U1].rearrange("p (k i) -> p k i", k=KH),
                    in0=z1[:].unsqueeze(2).to_broadcast([P, KH, IN_C]),
                    in1=xss1[:, t, :].unsqueeze(1).to_broadcast([P, KH, IN_C]),
                    op=AL.mult,
                )
                nc.scalar.copy(out=u[:, U1 : U1 + IN_C], in_=xss1[:, t, :])
                u_tiles.append(u)

            # scatter(PE) -> cast(Scalar) -> agg(PE), chunk-pipelined
            pre = pp.tile([P, H1], f32, tag="pre", bufs=2)
            uts_tiles = []
            wc1 = [P] * (NC1 - 1) + [U1R - (NC1 - 1) * P]
            for c in range(NC1):
                UT = pp.tile([P, P], f32, tag="ut", bufs=2)
                for t3 in range(T):
                    nc.tensor.matmul(
                        out=UT[: wc1[c], :],
                        lhsT=u_tiles[t3][:, c * P : c * P + wc1[c]],
                        rhs=oh[:, w * T + t3, :],
                        start=(t3 == 0), stop=(t3 == T - 1),
                    )
                uts = sb.tile([P, P], bf16, tag="uts")
                nc.scalar.copy(out=uts[: wc1[c], :], in_=UT[: wc1[c], :])
                uts_tiles.append(uts)
                if c > 0:
                    nc.tensor.matmul(out=pre[:], lhsT=uts_tiles[c - 1][: wc1[c - 1], :],
                                     rhs=Wp1[: wc1[c - 1], c - 1, :],
                                     start=(c - 1 == 0), stop=False)
            nc.tensor.matmul(out=pre[:], lhsT=uts_tiles[NC1 - 1][: wc1[NC1 - 1], :],
                             rhs=Wp1[: wc1[NC1 - 1], NC1 - 1, :],
                             start=False, stop=False)
            nc.tensor.matmul(out=pre[:], lhsT=xT[:, w * P : (w + 1) * P],
                             rhs=root1[:], start=False, stop=False)
            nc.tensor.matmul(out=pre[:], lhsT=onesrb[:], rhs=bias1r[:],
                             start=False, stop=True)
            # relu + pad mask, straight to bf16; pack [h1b | h1b^2] so one
            # matmul accumulates both BN1 stats in a single PSUM group
            hq = stash.tile([P, 2 * H1], bf16, tag="h1b")
            nc.vector.tensor_scalar(out=hq[:, :H1], in0=pre[:], scalar1=0.0,
                                    scalar2=vmaskw[:, w : w + 1],
                                    op0=AL.max, op1=AL.mult)
            nc.vector.tensor_mul(out=hq[:, H1:], in0=hq[:, :H1], in1=hq[:, :H1])
            h1b_list.append(hq)
            st1 = (pp.tile([1, 2 * H1], f32, tag="st1", name="st1")
                   if w == 0 else st1)  # noqa: F821
            nc.tensor.matmul(out=st1[:], lhsT=onesPb[:], rhs=hq[:],
                             start=(w == 0), stop=(w == WPC - 1))
            nc.sync.dma_start(out=h1_slice[w * P : (w + 1) * P, :], in_=hq[:, :H1])
            if DBG and w == 0:
                hqf = sb.tile([P, 2 * H1], f32, tag="hqf")
                nc.vector.tensor_copy(out=hqf[:], in_=hq[:])
                nc.sync.dma_start(out=dbg_h1[:], in_=hqf[:])

        # ---- stats rows (hi/lo bf16 pairs) ride along with the AllGather ----
        stf = sb.tile([1, 2 * H1], f32, tag="stf")
        nc.vector.tensor_copy(out=stf[:], in_=st1[:])
        sthi = sb.tile([1, 2 * H1], bf16, tag="sthi")
        nc.vector.tensor_copy(out=sthi[:], in_=stf[:])
        sthif = sb.tile([1, 2 * H1], f32, tag="sthif")
        nc.vector.tensor_copy(out=sthif[:], in_=sthi[:])
        stlo = sb.tile([1, 2 * H1], bf16, tag="stlo")
        nc.vector.tensor_sub(out=stlo[:], in0=stf[:], in1=sthif[:])
        nc.sync.dma_start(out=h1_slice[NPADC : NPADC + 1, :], in_=sthi[:, :H1])
        nc.sync.dma_start(out=h1_slice[NPADC + 1 : NPADC + 2, :], in_=sthi[:, H1:])
        nc.sync.dma_start(out=h1_slice[NPADC + 2 : NPADC + 3, :], in_=stlo[:, :H1])
        nc.sync.dma_start(out=h1_slice[NPADC + 3 : NPADC + 4, :], in_=stlo[:, H1:])
        nc.gpsimd.collective_compute(
            "AllGather", mybir.AluOpType.bypass, replica_groups=groups,
            ins=[h1_slice.opt()], outs=[h1_full.opt()],
        )

        # ---- overlapped with the AllGather: L2 edge-MLP + h1 transposes ----
        # z2all gets a trailing ones column so the xss block of u comes out of
        # the same broadcast-multiply as the z (x) xss outer product
        z2all = cst.tile([P, TT, KH + 1], bf16, tag="z2all")
        nc.vector.memset(z2all[:], 1.0)
        for t in range(TT):
            zp = pp.tile([P, KH], f32, tag="z", bufs=2)
            nc.tensor.matmul(out=zp[:], lhsT=eaT[:, t * P : (t + 1) * P],
                             rhs=W1a2[:], start=True, stop=True)
            nc.vector.tensor_scalar_max(out=z2all[:, t, :KH], in0=zp[:], scalar1=0.0)
        h1T_list = []
        for w in range(WPC):
            tp = pp.tile([P, P], bf16, tag="ut", bufs=2, name="tp")
            nc.tensor.transpose(out=tp[:H1, :], in_=h1b_list[w][:, :H1],
                                identity=identb[:])
            h1T = stash.tile([H1, P], bf16, tag="h1T")
            nc.scalar.copy(out=h1T[:], in_=tp[:H1, :])
            h1T_list.append(h1T)
        # pool one-hot (igc baked in) + vmask column, also during the gather
        ohgv = cst.tile([P, WPC, NG + 1], bf16, tag="ohgv")
        for w in range(WPC):
            nc.vector.tensor_scalar(
                out=ohgv[:, w, :NG], in0=iota256[:],
                scalar1=batchrelw[:, w : w + 1], scalar2=igcww[:, w : w + 1],
                op0=AL.is_equal, op1=AL.mult,
            )
            nc.scalar.copy(out=ohgv[:, w, NG:], in_=vmaskb[:, w : w + 1])

        # ---- BN1 coefficients from the gathered stats ----
        sgat = sb.tile([1, NCORES, 4 * H1], bf16, tag="sgat")
        for c in range(NCORES):
            r0c = c * NSTR + NPADC
            nc.sync.dma_start(
                out=sgat[:, c, :],
                in_=h1_full[r0c : r0c + 4, :].rearrange("r f -> (r f)").unsqueeze(0),
            )
        acc = sb.tile([1, 4 * H1], f32, tag="sacc")
        nc.vector.tensor_add(out=acc[:], in0=sgat[:, 0, :], in1=sgat[:, 1, :])
        for c in range(2, NCORES):
            nc.vector.tensor_add(out=acc[:], in0=acc[:], in1=sgat[:, c, :])
        tot = sb.tile([1, 2 * H1], f32, tag="tot")
        nc.vector.tensor_add(out=tot[:], in0=acc[:, : 2 * H1], in1=acc[:, 2 * H1 :])
        if DBG:
            nc.sync.dma_start(out=dbg_st[:], in_=acc[:])
        # tot = [sum | sumsq]; mu/var/scale/shift (row [1, H1])
        mu = sb.tile([1, H1], f32, tag="mu")
        nc.vector.tensor_scalar_mul(out=mu[:], in0=tot[:, :H1], scalar1=1.0 / N)
        va = sb.tile([1, H1], f32, tag="va")
        nc.vector.tensor_scalar_mul(out=va[:], in0=tot[:, H1:], scalar1=1.0 / N)
        musq = sb.tile([1, H1], f32, tag="musq")
        nc.vector.tensor_mul(out=musq[:], in0=mu[:], in1=mu[:])
        nc.vector.tensor_sub(out=va[:], in0=va[:], in1=musq[:])
        nc.vector.tensor_scalar_add(out=va[:], in0=va[:], scalar1=EPS)
        sd = sb.tile([1, H1], f32, tag="sd")
        nc.scalar.sqrt(out=sd[:], in_=va[:])
        rs = sb.tile([1, H1], f32, tag="rs")
        nc.vector.reciprocal(out=rs[:], in_=sd[:])
        sc_r = sb.tile([1, H1], f32, tag="sc_r")
        nc.vector.tensor_mul(out=sc_r[:], in0=rs[:], in1=bng1r[:])
        sh_r = sb.tile([1, H1], f32, tag="sh_r")
        nc.vector.tensor_mul(out=sh_r[:], in0=mu[:], in1=sc_r[:])
        nc.vector.tensor_sub(out=sh_r[:], in0=bnb1r[:], in1=sh_r[:])
        # column coeffs [H1, 1]; fold BN1 into root2/bias2 (root term) and
        # into a Wp2 scale plus rank-(KH+1) V correction (message term)
        sccp = pp.tile([H1, 1], f32, tag="z", bufs=2, name="sccp")
        nc.tensor.transpose(out=sccp[:], in_=sc_r[:], identity=onesr[:, :1])
        sc_c = sb.tile([H1, 1], f32, tag="sc_c")
        nc.vector.tensor_copy(out=sc_c[:], in_=sccp[:])
        sc_cb = sb.tile([H1, 1], bf16, tag="sc_cb")
        nc.vector.tensor_copy(out=sc_cb[:], in_=sccp[:])
        shcp = pp.tile([H1, 1], f32, tag="z", bufs=2, name="shcp")
        nc.tensor.transpose(out=shcp[:], in_=sh_r[:], identity=onesr[:, :1])
        sh_cb = sb.tile([H1, 1], bf16, tag="sh_cb")
        nc.vector.tensor_copy(out=sh_cb[:], in_=shcp[:])
        if DBG:
            bnrow = sb.tile([1, 2 * H1], f32, tag="bnrow")
            nc.vector.tensor_copy(out=bnrow[:, :H1], in_=sc_r[:])
            nc.vector.tensor_copy(out=bnrow[:, H1:], in_=sh_r[:])
            nc.sync.dma_start(out=dbg_bn[:], in_=bnrow[:])
        root2e = cst.tile([H1, H2], bf16, tag="root2e")
        nc.vector.tensor_scalar_mul(out=root2e[:], in0=root2[:], scalar1=sc_c[:, :1])
        b2p = pp.tile([1, H2], f32, tag="pre", bufs=2)
        nc.tensor.matmul(out=b2p[:], lhsT=sh_cb[:], rhs=root2[:], start=True, stop=True)
        bias2e = cst.tile([1, H2], bf16, tag="bias2e")
        nc.vector.tensor_add(out=bias2e[:], in0=b2p[:], in1=bias2r[:])
        # scv2[p] = sc[p % 64]; Wp2sc = scv2 * Wp2
        scvp = pp.tile([P, 1], f32, tag="z", bufs=2, name="scvp")
        nc.tensor.matmul(out=scvp[:], lhsT=dup64[:], rhs=sc_cb[:], start=True, stop=True)
        scv2 = sb.tile([P, 1], f32, tag="scv2")
        nc.vector.tensor_copy(out=scv2[:], in_=scvp[:])
        Wp2sc = cst.tile([P, NC2, H2], bf16, tag="Wp2sc")
        nc.vector.tensor_scalar_mul(out=Wp2sc[:], in0=Wp2[:], scalar1=scv2[:, :1])
        # V[k, o] = sum_i sh_i * W2[k, i, o]  (k = KH z-rows + the b2 row),
        # computed as one [1, 4224] row then reshaped to [33, 128] through DRAM
        NV = (KH + 1) * H2
        vrow = sb.tile([1, NV], bf16, tag="vrow")
        for q0 in range(0, NV, 512):
            q1 = min(q0 + 512, NV)
            vps = pp.tile([1, 512], f32, tag="z", bufs=2, name="vps")
            nc.tensor.matmul(out=vps[:, : q1 - q0], lhsT=sh_cb[:],
                             rhs=W2kx[:, q0:q1], start=True, stop=True)
            nc.scalar.copy(out=vrow[:, q0:q1], in_=vps[:, : q1 - q0])
        vd = dram.tile([KH + 1, H2], bf16, tag="vd")
        nc.sync.dma_start(
            out=vd[:].rearrange("k o -> (k o)").unsqueeze(0), in_=vrow[:]
        )
        V_sb = sb.tile([KH + 1, H2], bf16, tag="V_sb")
        nc.sync.dma_start(out=V_sb[:], in_=vd[:])

        # ==================== layer 2 ====================
        gTp = pp.tile([H2, NG + 1], f32, tag="g", name="gTp")
        gsq = pp.tile([H2, 1], f32, tag="st1", name="gsq")
        for w in range(WPC):
            u_tiles = []
            zic_tiles = []
            for t3 in range(T):
                t = w * T + t3
                xs = sb.tile([P, H1], bf16, tag="xs", bufs=UBUFS)
                nc.gpsimd.indirect_dma_start(
                    out=xs[:], out_offset=None, in_=h1_full[:],
                    in_offset=IndirectOffsetOnAxis(ap=srch[:, t : t + 1], axis=0),
                )
                if DBG and t == 0:
                    xsf = sb.tile([P, H1], f32, tag="xsf")
                    nc.vector.tensor_copy(out=xsf[:], in_=xs[:])
                    nc.sync.dma_start(out=dbg_xs[:], in_=xsf[:])
                    z2f = sb.tile([P, KH], f32, tag="z2f")
                    nc.vector.tensor_copy(out=z2f[:], in_=z2all[:, 0, :])
                    nc.sync.dma_start(out=dbg_z[:], in_=z2f[:])
                xss = sb.tile([P, H1], bf16, tag="xss", bufs=UBUFS)
                nc.vector.tensor_scalar_mul(out=xss[:], in0=xs[:],
                                            scalar1=icnt[:, t : t + 1])
                zic = sb.tile([P, KH + 1], bf16, tag="zic", bufs=UBUFS)
                nc.vector.tensor_scalar_mul(out=zic[:], in0=z2all[:, t, :],
                                            scalar1=icnt[:, t : t + 1])
                zic_tiles.append(zic)
                u = sb.tile([P, U2R], bf16, tag="u2", bufs=UBUFS)
                nc.vector.tensor_tensor(
                    out=u[:, : KV2 * H1].rearrange("p (k i) -> p k i", k=KV2),
                    in0=z2all[:, t, :KV2].unsqueeze(2).to_broadcast([P, KV2, H1]),
                    in1=xss[:].unsqueeze(1).to_broadcast([P, KV2, H1]),
                    op=AL.mult,
                )
                nc.gpsimd.tensor_tensor(
                    out=u[:, KV2 * H1 : U2R].rearrange("p (k i) -> p k i",
                                                       k=KH + 1 - KV2),
                    in0=z2all[:, t, KV2:].unsqueeze(2).to_broadcast(
                        [P, KH + 1 - KV2, H1]),
                    in1=xss[:].unsqueeze(1).to_broadcast([P, KH + 1 - KV2, H1]),
                    op=AL.mult,
                )
                if DBG and t == 0:
                    xssf = sb.tile([P, H1], f32, tag="xssf")
                    nc.vector.tensor_copy(out=xssf[:], in_=xss[:])
                    nc.sync.dma_start(out=dbg_xss[:], in_=xssf[:])
                    uf = sb.tile([P, H1], f32, tag="uf")
                    nc.vector.tensor_copy(out=uf[:], in_=u[:, :H1])
                    nc.sync.dma_start(out=dbg_u[:], in_=uf[:])
                u_tiles.append(u)

            pre = pp.tile([P, H2], f32, tag="pre", bufs=2)
            uts_tiles = []
            wc2 = [P] * (NC2 - 1) + [U2R - (NC2 - 1) * P]
            for c in range(NC2):
                UT = pp.tile([P, P], f32, tag="ut", bufs=2)
                for t3 in range(T):
                    nc.tensor.matmul(
                        out=UT[: wc2[c], :],
                        lhsT=u_tiles[t3][:, c * P : c * P + wc2[c]],
                        rhs=oh[:, w * T + t3, :],
                        start=(t3 == 0), stop=(t3 == T - 1),
                    )
                uts = sb.tile([P, P], bf16, tag="uts")
                nc.scalar.copy(out=uts[: wc2[c], :], in_=UT[: wc2[c], :])
                if DBG and w == 0 and c == 0:
                    utsf = sb.tile([P, P], f32, tag="utsf")
                    nc.vector.tensor_copy(out=utsf[:], in_=uts[:])
                    nc.sync.dma_start(out=dbg_uts[:], in_=utsf[:])
                uts_tiles.append(uts)
                if c > 0:
                    nc.tensor.matmul(out=pre[:], lhsT=uts_tiles[c - 1][: wc2[c - 1], :],
                                     rhs=Wp2sc[: wc2[c - 1], c - 1, :],
                                     start=(c - 1 == 0), stop=False)
            # rank-(KH+1) shift correction: scatter zic, multiply by V
            UZ = pp.tile([KH + 1, P], f32, tag="z", bufs=2, name="UZ")
            for t3 in range(T):
                nc.tensor.matmul(out=UZ[:], lhsT=zic_tiles[t3][:],
                                 rhs=oh[:, w * T + t3, :],
                                 start=(t3 == 0), stop=(t3 == T - 1))
            uzs = sb.tile([KH + 1, P], bf16, tag="uzs")
            nc.scalar.copy(out=uzs[:], in_=UZ[:])
            nc.tensor.matmul(out=pre[:], lhsT=uts_tiles[NC2 - 1][: wc2[NC2 - 1], :],
                             rhs=Wp2sc[: wc2[NC2 - 1], NC2 - 1, :],
                             start=False, stop=False)
            nc.tensor.matmul(out=pre[:], lhsT=uzs[:], rhs=V_sb[:],
                             start=False, stop=False)
            nc.tensor.matmul(out=pre[:], lhsT=h1T_list[w][:], rhs=root2e[:],
                             start=False, stop=False)
            nc.tensor.matmul(out=pre[:], lhsT=onesrb[:], rhs=bias2e[:],
                             start=False, stop=True)
            h2b = sb.tile([P, H2], bf16, tag="h2b")
            nc.vector.tensor_scalar(out=h2b[:], in0=pre[:], scalar1=0.0,
                                    scalar2=vmaskw[:, w : w + 1],
                                    op0=AL.max, op1=AL.mult)
            sqb2 = sb.tile([P, H2], bf16, tag="sqb2")
            nc.vector.tensor_mul(out=sqb2[:], in0=h2b[:], in1=h2b[:])
            # pool (with igc baked into ohgv) + BN2 sum stat in one matmul;
            # sumsq stat rides as one more column
            nc.tensor.matmul(out=gTp[:], lhsT=h2b[:], rhs=ohgv[:, w, :],
                             start=(w == 0), stop=(w == WPC - 1))
            nc.tensor.matmul(out=gsq[:], lhsT=sqb2[:],
                             rhs=vmaskb[:, w : w + 1],
                             start=(w == 0), stop=(w == WPC - 1))
            if DBG and w == 0:
                h2f = sb.tile([P, H2], f32, tag="h2f")
                nc.vector.tensor_copy(out=h2f[:], in_=h2b[:])
                nc.sync.dma_start(out=dbg_h2[:], in_=h2f[:])

        # ---- one AllReduce carries pooled graph features + BN2 stats ----
        fin_sb = sb.tile([P, NG + 2], f32, tag="fin")
        nc.vector.tensor_copy(out=fin_sb[:, : NG + 1], in_=gTp[:])
        nc.vector.tensor_copy(out=fin_sb[:, NG + 1 :], in_=gsq[:])
        nc.sync.dma_start(out=fin_loc[:], in_=fin_sb[:])
        if DBG:
            fin_dbg = sb.tile([P, NG + 2], f32, tag="fin_dbg")
            nc.vector.tensor_copy(out=fin_dbg[:, : NG + 1], in_=gTp[:])
            nc.vector.tensor_copy(out=fin_dbg[:, NG + 1 :], in_=gsq[:])
            nc.sync.dma_start(out=dbg_finl[:], in_=fin_dbg[:])
        nc.gpsimd.collective_compute(
            "AllReduce", mybir.AluOpType.add, replica_groups=groups,
            ins=[fin_loc.opt()], outs=[fin_g.opt()],
        )
        fin = sb.tile([P, NG + 2], f32, tag="fin2")
        nc.sync.dma_start(out=fin[:], in_=fin_g[:])
        # bn2 coeffs (column orientation [H2, 1])
        mu2 = sb.tile([H2, 1], f32, tag="mu2")
        nc.vector.tensor_scalar_mul(out=mu2[:], in0=fin[:, NG : NG + 1], scalar1=1.0 / N)
        va2 = sb.tile([H2, 1], f32, tag="va2")
        nc.vector.tensor_scalar_mul(out=va2[:], in0=fin[:, NG + 1 : NG + 2], scalar1=1.0 / N)
        musq2 = sb.tile([H2, 1], f32, tag="musq2")
        nc.vector.tensor_mul(out=musq2[:], in0=mu2[:], in1=mu2[:])
        nc.vector.tensor_sub(out=va2[:], in0=va2[:], in1=musq2[:])
        nc.vector.tensor_scalar_add(out=va2[:], in0=va2[:], scalar1=EPS)
        sd2 = sb.tile([H2, 1], f32, tag="sd2")
        nc.scalar.sqrt(out=sd2[:], in_=va2[:])
        rs2 = sb.tile([H2, 1], f32, tag="rs2")
        nc.vector.reciprocal(out=rs2[:], in_=sd2[:])
        sc2 = sb.tile([H2, 1], f32, tag="sc2")
        nc.vector.tensor_mul(out=sc2[:], in0=rs2[:], in1=bng2[:])
        sh2 = sb.tile([H2, 1], f32, tag="sh2")
        nc.vector.tensor_mul(out=sh2[:], in0=mu2[:], in1=sc2[:])
        nc.vector.tensor_sub(out=sh2[:], in0=bnb2[:], in1=sh2[:])
        # g = sc2 * g_raw + sh2   (BN2 folded through the pool; every graph in
        # this workload is non-empty, so the empty-graph mask is the constant 1)
        gt = sb.tile([P, NG], f32, tag="gt")
        nc.vector.tensor_scalar(out=gt[:], in0=fin[:, :NG], scalar1=sc2[:, :1],
                                scalar2=sh2[:, :1], op0=AL.mult, op1=AL.add)

        # ---- final MLP ----
        l1p = pp.tile([H1, NG], f32, tag="g", name="l1p")
        nc.tensor.matmul(out=l1p[:], lhsT=l1W[:], rhs=gt[:], start=True, stop=True)
        hl = sb.tile([H1, NG], f32, tag="hl")
        nc.vector.tensor_scalar(out=hl[:], in0=l1p[:], scalar1=l1b[:, :1],
                                scalar2=0.0, op0=AL.add, op1=AL.max)
        l2pf = pp.tile([1, NG], f32, tag="pre", bufs=2, name="l2pf")
        nc.tensor.matmul(out=l2pf[:], lhsT=l2W[:], rhs=hl[:], start=True, stop=True)
        osb = sb.tile([1, NG], f32, tag="osb")
        nc.vector.tensor_scalar_add(out=osb[:], in0=l2pf[:], scalar1=l2b[:, :1])
        nc.sync.dma_start(out=out_d[:], in_=osb[:])

    nc.compile()
    return nc


_CACHE = {}


def _get_program(T, ES):
    key = (T, ES)
    if key not in _CACHE:
        _CACHE[key] = build_program(T, ES)
    return _CACHE[key]


def make_in_maps(inputs):
    pp = _preprocess(
        inputs["x"], inputs["edge_index"], inputs["edge_attr"], inputs["batch"]
    )
    w = _weights(inputs)
    shared = dict(
        W1a1=w["W1a1"], W1a2=w["W1a2"], Wp1=w["Wp1"], Wp2=w["Wp2"],
        root1=w["root1"], root2=w["root2"], bias1r=w["bias1r"], bias2r=w["bias2r"],
        bng1r=w["bng1r"], bnb1r=w["bnb1r"], bng2=w["bng2"], bnb2=w["bnb2"],
        l1W=w["l1W"], l1b=w["l1b"], l2W=w["l2W"], l2b=w["l2b"],
        identb=w["identb"], onesr=w["onesr"], onesrb=w["onesrb"], onesPb=w["onesPb"],
        iota128=w["iota128"], iota256=w["iota256"],
        W2kx=w["W2kx"], dup64=w["dup64"],
    )
    in_maps = []
    for c in range(NCORES):
        m = dict(shared)
        m["eaT"] = np.ascontiguousarray(pp["eaT"][c])
        m["xss1"] = pp["xss1"][c]
        m["drel"] = pp["drel"][c]
        m["srch"] = np.ascontiguousarray(
            pp["srch"][c].reshape(pp["TT"], P).T
        )
        m["icnt"] = np.ascontiguousarray(
            pp["icnt"][c].reshape(pp["TT"], P).T
        )
        m["xT"] = np.ascontiguousarray(pp["xT"][c].astype(bf))
        m["vmaskw"] = pp["vmaskw"][c]
        m["vmaskb"] = pp["vmaskb"][c]
        m["batchrelw"] = pp["batchrelw"][c]
        m["igcww"] = pp["igcww"][c]
        in_maps.append(m)
    return in_maps, pp["T"], pp["ES"]


def _run(inputs, trace=False):
    in_maps, T, ES = make_in_maps(inputs)
    nc = _get_program(T, ES)
    res = run_bass_kernel_spmd(
        nc, in_maps, core_ids=list(range(NCORES)), trace=trace
    )
    out = np.asarray(res.results[0]["out"][0], dtype=np.float32)
    return out, res


def kernel(**inputs):
    return _run(inputs)[0]
